# revision 54
# baseline (speedup 1.0000x reference)
"""Trainium2 Bass kernel for nn_Attention_80384607912675.

Multi-head attention (B=2, S=2048, D=1024, H=16, HD=64), fp32 reference.

Sharding (8 cores): data-parallel over batch (2) x tensor-parallel over heads
(4 head groups of 4 heads).  Core c handles batch c//4, heads [4*(c%4), 4*(c%4)+4).
wq/wk/wv split column-wise, wo split row-wise; the wo partial sums (and the
bias bo) are reduced on the host in fp32.

Per-core kernel (all matmuls bf16, fp32 PSUM accumulation):
  QT/KT = (x @ wq/k + b)^T   head-major [128 (2 heads x 64), 2048] per pair
  V     = x @ wv + bv        natural    [2048, 256] (xT as lhsT -> no transpose)
  per window w = (head pair hp, 512-wide q window qw), kp-chunk c:
    S^T[kp, (h, q)] = K_h^T (x) Q_h      packed [128, 1024] PSUM (A|B)
    P^T             = exp(S^T / 8)       one ACT instr -> bf16 SBUF
    O[q, (qc,h,hd)] += P^T(x)V chunks    [128 q, 64] tiles, full-partition PE
    rs[q, (qc,h)]   += P^T(x)1           rowsums, 1-wide matmuls
  drain: O+rs PSUM -> SBUF copy (frees the single O bank fast), reciprocal,
  per-partition normalize mul, PE transpose to O^T, out = O^T.T @ wo_c
  -> bf16 [2048, 1024] partial, DMA per [128, 512] tile.

Timing model notes (TimelineSim): PE matmul cost = out free size x 0.42ns x
k-chunks, independent of M/K utilization -- so PV uses full 128 q-partitions
(half the cost of the [65, 1024] O^T layout) and rowsums are near-free 1-wide
matmuls.  ACT exp is the second-closest engine to the roofline (~133us busy);
PE is the critical engine (~144us).  The schedule is exp-slot based: per slot
one sc QK pair + one exp, with PV lagged ~12 slots behind (the single PSUM O
bank serializes windows via the drain), and all projection/V/outproj work
packed greedily into per-slot PE budget via a FIFO unit queue with
earliest-slot gates and force-emit deadlines.  CRITICAL INVARIANT: a unit
producing data read at slot g must have deadline <= g-1, because slot g's
QK/exp are emitted first and emission order defines dependency direction in
the tile framework.  Dummy identity transposes burn the PE pstate ramp
(full speed needs ~3us of continuous execution) during the input-DMA wait;
xT streams in two s-halves so lead projections start ~5us in; window 7
drains directly from PSUM with normalize split ACT/DVE to shorten the tail.

Verified: CoreSim core-0 max err 1.7e-3 vs fp64 numpy; full 8-core test
rel err 5.3e-3 (gate 2e-2).  207.1us (baseline) -> 167.6us (-19%).
"""

import numpy as np

B, S, D, H = 2, 2048, 1024, 16
HD = D // H          # 64
HPC = 4              # heads per core
DHC = HPC * HD       # 256 head dims per core
KC = D // 128        # 8 contraction chunks
SB = S // 128        # 16 s blocks / kp chunks
NC = 8               # cores
NW = 8               # windows (2 head pairs x 4 q windows)

_nc_cache = {}


def _build_bass(with_bias=False):
    import concourse.mybir as mybir
    import concourse.tile as tile
    from concourse import bacc
    from concourse.masks import make_identity

    BF = mybir.dt.bfloat16
    F32 = mybir.dt.float32
    EXP = mybir.ActivationFunctionType.Exp

    nc = bacc.Bacc("TRN2")

    xT_d = nc.dram_tensor("xT", [D, S], BF, kind="ExternalInput")
    wq_d = nc.dram_tensor("wq_c", [D, DHC], BF, kind="ExternalInput")
    wk_d = nc.dram_tensor("wk_c", [D, DHC], BF, kind="ExternalInput")
    wv_d = nc.dram_tensor("wv_c", [D, DHC], BF, kind="ExternalInput")
    wo_d = nc.dram_tensor("wo_c", [DHC, D], BF, kind="ExternalInput")
    bias_d = nc.dram_tensor("bias3", [1, 3 * DHC], BF, kind="ExternalInput")
    out_d = nc.dram_tensor("out", [S, D], BF, kind="ExternalOutput")

    with tile.TileContext(nc) as tc:
        with (
            tc.tile_pool(name="persist", bufs=1) as pp,
            tc.tile_pool(name="sc", bufs=2, space="PSUM") as scp,
            tc.tile_pool(name="oacc", bufs=1, space="PSUM") as opp,
            tc.tile_pool(name="pj", bufs=2, space="PSUM") as pjp,
            tc.tile_pool(name="pt", bufs=12) as ptp,
            tc.tile_pool(name="osb", bufs=2) as osbp,
            tc.tile_pool(name="nrm", bufs=2) as nrmp,
            tc.tile_pool(name="rc", bufs=2) as rcp,
            tc.tile_pool(name="ot", bufs=8) as otp,
        ):
            xT_sb = pp.tile([128, KC * S], BF, tag="xT", name="xT_sb")
            wq_sb = pp.tile([128, KC * DHC], BF, tag="wq", name="wq_sb")
            wk_sb = pp.tile([128, KC * DHC], BF, tag="wk", name="wk_sb")
            wv_sb = pp.tile([128, KC * DHC], BF, tag="wv", name="wv_sb")
            wo_sb = pp.tile([128, 2 * D], BF, tag="wo", name="wo_sb")
            qt_sb = pp.tile([128, 2 * S], BF, tag="qt", name="qt_sb")
            kt_sb = pp.tile([128, 2 * S], BF, tag="kt", name="kt_sb")
            v_sb = pp.tile([128, SB * DHC], BF, tag="v", name="v_sb")
            onmT_sb = pp.tile([128, 2 * S], BF, tag="onmT", name="onmT_sb")
            ident = pp.tile([128, 128], BF, tag="ident", name="ident")
            bias_sb = pp.tile([1, 3 * DHC], BF, tag="bias", name="bias_sb")
            ones16 = pp.tile([1, 512], BF, tag="ones16", name="ones16")
            ones_col = pp.tile([128, 1], BF, tag="ones_col", name="ones_col")

            # ---- input DMAs: small weights first, xT streamed in s-halves
            # (16 DMAs) so nt0/nt1-dependent projections start early; wo last.
            def load_w(w_sb, w_d):
                nc.sync.dma_start(
                    w_sb[:, :].rearrange("p (k d) -> p k d", d=DHC),
                    w_d[:, :].rearrange("(k p) d -> p k d", p=128),
                )

            def load_xt(k, h):
                nc.sync.dma_start(
                    xT_sb[:, k * S + h * 1024: k * S + (h + 1) * 1024],
                    xT_d[k * 128:(k + 1) * 128, h * 1024:(h + 1) * 1024],
                )

            load_w(wk_sb, wk_d)
            load_xt(0, 0)
            load_xt(1, 0)
            load_w(wq_sb, wq_d)
            for k in range(2, KC):
                load_xt(k, 0)
            load_w(wv_sb, wv_d)
            for k in range(KC):
                load_xt(k, 1)
            nc.sync.dma_start(bias_sb[:, :], bias_d[:, :])
            nc.sync.dma_start(
                wo_sb[:, :].rearrange("r (p d) -> r p d", d=D),
                wo_d[:, :].rearrange("(p r) d -> r p d", r=128),
            )
            nc.vector.memset(ones16[:, :], 1.0)
            nc.vector.memset(ones_col[:, :], 1.0)
            make_identity(nc, ident[:, :])

            bq = bias_sb[0:1, 0:DHC]
            bk = bias_sb[0:1, DHC:2 * DHC]
            bv = bias_sb[0:1, 2 * DHC:3 * DHC]

            # ---- Q/K projection tiles (p: head-pair block, nt: 512 s cols),
            # emitted in four 2-k-chunk quarters so no single filler slot
            # exceeds the exp budget (locally PE-stalled exp slots are never
            # recovered).
            pend = {}

            def qk_quarter(dst_sb, w_sb, bias, p, nt, q, eng="dve"):
                key = (dst_sb.tensor.name, p, nt)
                if q == 0:
                    ps = pjp.tile([128, 512], F32, tag="pj",
                                  name=f"qk_{key[0]}_{p}_{nt}")
                    pend[key] = ps
                else:
                    ps = pend[key]
                for k in (2 * q, 2 * q + 1):
                    nc.tensor.matmul(
                        ps[:, :],
                        lhsT=w_sb[:, k * DHC + p * 128: k * DHC + (p + 1) * 128],
                        rhs=xT_sb[:, k * S + nt * 512: k * S + (nt + 1) * 512],
                        start=(k == 0),
                        stop=(k == KC - 1 and not with_bias),
                    )
                if q == 3:
                    del pend[key]
                    if with_bias:
                        nc.tensor.matmul(
                            ps[:, :],
                            lhsT=bias[:, p * 128:(p + 1) * 128],
                            rhs=ones16[0:1, :],
                            start=False, stop=True,
                        )
                    dst = dst_sb[:, p * S + nt * 512: p * S + (nt + 1) * 512]
                    if eng == "act":
                        nc.scalar.copy(dst, ps[:, :])
                    else:
                        nc.vector.tensor_copy(dst, ps[:, :])

            # ---- V pair tiles: pair j covers s-chunks 2j, 2j+1 in natural
            # layout (lhsT = xT s-slice, rhs = wv chunk), in 4-matmul
            # quarters.  Pairs 0/1 run in the lead psum slots (lv).
            vpend = {}

            def v_bias_mms(ps):
                for h2 in range(2):
                    nc.tensor.matmul(
                        ps[:, h2 * 256:(h2 + 1) * 256],
                        lhsT=ones16[0:1, 0:128], rhs=bv[:, :],
                        start=False, stop=(h2 == 1),
                    )

            def v_quarter(j, q):
                if j < 2:
                    ps = lv[j]
                elif q == 0:
                    ps = pjp.tile([128, 512], F32, tag="pj", name=f"v_{j}")
                    vpend[j] = ps
                else:
                    ps = vpend[j]
                half = q // 2
                sc_ = 2 * j + half
                for k in range(4 * (q % 2), 4 * (q % 2) + 4):
                    nc.tensor.matmul(
                        ps[:, half * 256:(half + 1) * 256],
                        lhsT=xT_sb[:, k * S + sc_ * 128: k * S + (sc_ + 1) * 128],
                        rhs=wv_sb[:, k * DHC: k * DHC + DHC],
                        start=(k == 0 and half == 0),
                        stop=(k == KC - 1 and half == 1 and not with_bias),
                    )
                if q == 3:
                    vpend.pop(j, None)
                    if with_bias:
                        v_bias_mms(ps)
                    nc.vector.tensor_copy(
                        v_sb[:, 2 * j * DHC: (2 * j + 2) * DHC],
                        ps[:, 0:512])

            # ---- per-window attention pieces
            otiles = {}
            pts = {}

            def emit_qk_exp(w, c):
                hp, qw = w // 4, w % 4
                sc = scp.tile([128, 1024], F32, tag="sc", name=f"sc_{w}_{c}")
                for i in range(2):
                    nc.tensor.matmul(
                        sc[:, 512 * i:512 * (i + 1)],
                        lhsT=kt_sb[64 * i:64 * (i + 1),
                                   hp * S + c * 128: hp * S + (c + 1) * 128],
                        rhs=qt_sb[64 * i:64 * (i + 1),
                                  hp * S + qw * 512: hp * S + (qw + 1) * 512],
                        start=True, stop=True,
                    )
                pt = ptp.tile([128, 1024], BF, tag="pt", name=f"pt_{w}_{c}")
                nc.scalar.activation(pt[:, :], sc[:, :], EXP, scale=0.125)
                pts[(w, c)] = pt

            def pv(w, c):
                hp = w // 4
                if c == 0:
                    otiles[w] = opp.tile([128, 520], F32, tag="oacc",
                                         name=f"o_{w}")
                O = otiles[w]
                pt = pts.pop((w, c))
                for qc in range(4):
                    for h in range(2):
                        first = (c == 0 and qc == 0 and h == 0)
                        last = (c == SB - 1 and qc == 3 and h == 1)
                        lh = pt[:, h * 512 + qc * 128: h * 512 + (qc + 1) * 128]
                        nc.tensor.matmul(
                            O[:, qc * 128 + h * 64: qc * 128 + h * 64 + 64],
                            lhsT=lh,
                            rhs=v_sb[:, c * DHC + (2 * hp + h) * 64:
                                     c * DHC + (2 * hp + h) * 64 + 64],
                            start=first, stop=last,
                        )
                        nc.tensor.matmul(
                            O[:, 512 + qc * 2 + h: 513 + qc * 2 + h],
                            lhsT=lh, rhs=ones_col[:, 0:1],
                            start=first, stop=last,
                        )

            osbs = {}
            nrms = {}

            def drain_a(w):
                osb = osbp.tile([128, 520], F32, tag="osb", name=f"osb_{w}")
                nc.vector.tensor_copy(osb[:, :], otiles.pop(w)[:, :])
                osbs[w] = osb

            def drain_b(w):
                osb = osbs.pop(w)
                rc = rcp.tile([128, 8], F32, tag="rc", name=f"rc_{w}")
                nc.vector.reciprocal_approx_fast(
                    out=rc[:, :], in_=osb[:, 512:520])
                nrm = nrmp.tile([128, 512], BF, tag="nrm", name=f"nrm_{w}")
                for qc in range(4):
                    for h in range(2):
                        col = qc * 128 + h * 64
                        nc.vector.tensor_scalar_mul(
                            nrm[:, col:col + 64], osb[:, col:col + 64],
                            rc[:, qc * 2 + h: qc * 2 + h + 1])
                nrms[w] = nrm

            def drain_tp(w, qc):
                hp, qw = w // 4, w % 4
                nrm = nrms[w]
                tp = pjp.tile([128, 128], BF, tag="pj", name=f"tp_{w}_{qc}")
                nc.tensor.transpose(
                    tp[:, :], nrm[:, qc * 128:(qc + 1) * 128], ident[:, :])
                nc.vector.tensor_copy(
                    onmT_sb[:, hp * S + qw * 512 + qc * 128:
                            hp * S + qw * 512 + (qc + 1) * 128], tp[:, :])
                if qc == 3:
                    del nrms[w]

            def outproj(qw, qc, n, eng="dve"):
                t = qw * 4 + qc
                po = pjp.tile([128, 512], F32, tag="pj", name=f"po_{t}_{n}")
                for hp in range(2):
                    nc.tensor.matmul(
                        po[:, :],
                        lhsT=onmT_sb[:, hp * S + qw * 512 + qc * 128:
                                     hp * S + qw * 512 + (qc + 1) * 128],
                        rhs=wo_sb[:, hp * D + n * 512: hp * D + (n + 1) * 512],
                        start=(hp == 0), stop=(hp == 1),
                    )
                ot = otp.tile([128, 512], BF, tag="ot", name=f"ot_{t}_{n}")
                if eng == "act":
                    nc.scalar.copy(ot[:, :], po[:, :])
                else:
                    nc.vector.tensor_copy(ot[:, :], po[:, :])
                nc.sync.dma_start(
                    out_d[t * 128:(t + 1) * 128, n * 512:(n + 1) * 512],
                    ot[:, :])

            # ---- lead-in: kt/qt (p0, nt0) + V pairs 0,1 pipelined against
            # the arriving xT halves; kt/qt finish first so window 0 starts
            # as early as possible.  Dummy identity transposes keep the PE
            # continuously busy from t~0 so the pstate ramp (full speed after
            # 3us of uninterrupted execution) is burned during the input DMA
            # instead of doubling every lead matmul.
            def ramp(n):
                for _ in range(n):
                    nc.tensor.transpose(
                        dummy_bf[:, :], ident[:, :], ident[:, :])

            dummy_bf = pjp.tile([128, 128], BF, tag="pj", name="dummy_bf")
            lt = scp.tile([128, 1024], F32, tag="sc", name="lead_ktqt")
            lv = [opp.tile([128, 520], F32, tag="oacc", name="lead_v01"),
                  pjp.tile([128, 512], F32, tag="pj", name="lead_v23")]

            def lead_mm(k):
                for half, (w_sb,) in enumerate([(wk_sb,), (wq_sb,)]):
                    nc.tensor.matmul(
                        lt[:, half * 512:(half + 1) * 512],
                        lhsT=w_sb[:, k * DHC: k * DHC + 128],
                        rhs=xT_sb[:, k * S: k * S + 512],
                        start=(k == 0),
                        stop=(k == KC - 1 and not with_bias),
                    )

            def lead_vmm(k, pair):
                ps = lv[pair]
                for h2 in range(2):
                    sc_ = 2 * pair + h2
                    nc.tensor.matmul(
                        ps[:, h2 * 256:(h2 + 1) * 256],
                        lhsT=xT_sb[:, k * S + sc_ * 128: k * S + (sc_ + 1) * 128],
                        rhs=wv_sb[:, k * DHC: k * DHC + DHC],
                        start=(k == 0 and h2 == 0),
                        stop=(k == KC - 1 and h2 == 1 and not with_bias),
                    )

            ramp(15)
            for k in range(KC):
                lead_mm(k)
                if k < KC - 1:
                    ramp(2 if k < 4 else 5)
            if with_bias:
                for half, bias in enumerate([bk, bq]):
                    nc.tensor.matmul(
                        lt[:, half * 512:(half + 1) * 512],
                        lhsT=bias[:, 0:128], rhs=ones16[0:1, :],
                        start=False, stop=True,
                    )
            nc.scalar.copy(kt_sb[:, 0:512], lt[:, 0:512])
            nc.vector.tensor_copy(qt_sb[:, 0:512], lt[:, 512:1024])

            # ---- schedule ----
            # Mandatory per-slot items (PV cadence, drains, transposes) are
            # placed at fixed global slots g = 16*w + c.  Everything else
            # (projection/V quarters, outproj pieces) lives in a strict-FIFO
            # unit queue packed greedily against each slot's leftover PE
            # budget, with earliest-slot gates and force-emit deadlines.
            sched = {}
            mcost = {}

            def at(g, fn, cost=0):
                sched.setdefault(g, []).append(fn)
                mcost[g] = mcost.get(g, 0) + cost

            # PV cadence: PV(w, c) lags exp by ~12 slots; the single O bank
            # serializes windows (drain_a frees it).  Last two windows
            # compress so the tail stays short.
            for w in range(NW):
                for c in range(SB):
                    if w < 6:
                        g = 16 * w + 12 + c
                    elif w == 6:
                        g = 108 + c if c < 8 else 112 + (c - 8) // 2
                    else:
                        g = (120 + c if c < 4 else
                             124 + (c - 4) // 2 if c < 12 else 200)
                    at(g, lambda w=w, c=c: pv(w, c), 240)
            for w in range(NW - 1):
                if w < 6:
                    ga, gtp = 16 * w + 27, 16 * w + 30
                else:
                    ga, gtp = 115, 118
                at(ga, lambda w=w: drain_a(w))
                at(ga + 1, lambda w=w: drain_b(w))
                for qc in range(4):
                    at(gtp + qc // 2, lambda w=w, qc=qc: drain_tp(w, qc), 60)

            # filler unit queue
            units = []

            def unit(e, dls, cost, mk):
                for q, d in enumerate(dls):
                    units.append((e[q] if isinstance(e, list) else e, d, cost,
                                  mk(q)))

            def mk_qk(dst, wt, bias, p, nt, eng="dve"):
                return lambda q: (lambda: qk_quarter(dst, wt, bias, p, nt, q,
                                                     eng))

            def mk_v(j):
                return lambda q: (lambda: v_quarter(j, q))

            def mk_po(qw, qc, n, eng="dve"):
                return lambda q: (lambda: outproj(qw, qc, n, eng))

            unit(0, [0, 1, 2, 3], 430, mk_qk(kt_sb, wk_sb, bk, 0, 1, "act"))
            unit(1, [3, 4, 5, 6], 430, mk_v(0))
            unit([1, 2, 3, 5], [4, 5, 6, 7], 430,
                 mk_qk(kt_sb, wk_sb, bk, 0, 2, "act"))
            unit(1, [5, 6, 7, 8], 430, mk_v(1))
            unit(0, [8, 9, 10, 11], 430, mk_v(2))
            unit([1, 2, 3, 5], [8, 9, 10, 11], 430,
                 mk_qk(kt_sb, wk_sb, bk, 0, 3, "act"))
            unit(0, [10, 11, 12, 13], 430, mk_v(3))
            unit(0, [12, 13, 14, 15], 430, mk_qk(qt_sb, wq_sb, bq, 0, 1))
            unit([3, 5, 5, 5], [14, 15, 16, 17], 430, mk_v(4))
            unit([4, 5, 5, 6], [18, 19, 20, 21], 430, mk_v(5))
            unit([4, 5, 6, 6], [20, 21, 22, 23], 430, mk_v(6))
            unit([5, 6, 6, 7], [22, 23, 24, 25], 430, mk_v(7))
            unit(0, [28, 29, 30, 31], 430, mk_qk(qt_sb, wq_sb, bq, 0, 2))
            unit(0, [44, 45, 46, 47], 430, mk_qk(qt_sb, wq_sb, bq, 0, 3))
            unit(0, [60, 61, 62, 63], 430, mk_qk(kt_sb, wk_sb, bk, 1, 0))
            unit(0, [64, 65, 66, 67], 430, mk_qk(kt_sb, wk_sb, bk, 1, 1))
            unit(0, [68, 69, 70, 71], 430, mk_qk(kt_sb, wk_sb, bk, 1, 2))
            unit(0, [72, 73, 74, 75], 430, mk_qk(kt_sb, wk_sb, bk, 1, 3))
            unit(0, [60, 61, 62, 63], 430, mk_qk(qt_sb, wq_sb, bq, 1, 0))
            unit(0, [76, 77, 78, 79], 430, mk_qk(qt_sb, wq_sb, bq, 1, 1))
            unit([88, 89, 90, 91], [92, 93, 94, 95], 430, mk_qk(qt_sb, wq_sb, bq, 1, 2))
            for i in range(8):
                units.append((96, 118 + i, 430, (lambda i=i: outproj(
                    0, i // 2, i % 2))))
            unit([100, 101, 102, 103], [108, 109, 110, 111], 430, mk_qk(qt_sb, wq_sb, bq, 1, 3))
            for i in range(8):
                units.append((112, 113 + i, 430, (lambda i=i: outproj(
                    1, i // 2, i % 2))))
            for i in range(8):
                units.append((120, 120 + i, 430, (lambda i=i: outproj(
                    2, i // 2, i % 2))))

            # ---- main loop with budget packing
            uidx = 0
            for w in range(NW):
                for c in range(SB):
                    g = 16 * w + c
                    emit_qk_exp(w, c)
                    for fn in sched.get(g, ()):
                        fn()
                    budget = 1038 - 430 - mcost.get(g, 0)
                    spent = 0
                    while uidx < len(units):
                        e, d, cost, fn = units[uidx]
                        if e > g:
                            break
                        if (d <= g or spent + cost <= budget + 120
                                or (spent == 0 and budget >= 300)):
                            fn()
                            spent += cost
                            uidx += 1
                        else:
                            break

            # ---- tail: leftover units, then finish window 7.  onmT copies
            # (DVE) go ahead of the outproj staging copies; staging copies
            # alternate ACT/DVE.
            for fn in sched.get(200, ()):
                fn()
            # direct-from-PSUM drain of window 7: normalization runs on ACT
            # (idle after the last exp), per-qc so transposes and outproj
            # pipeline behind it.
            O7 = otiles.pop(7)
            rc7 = rcp.tile([128, 8], F32, tag="rc", name="rc_7")
            nc.vector.reciprocal_approx_fast(out=rc7[:, :], in_=O7[:, 512:520])
            nrm7 = nrmp.tile([128, 512], BF, tag="nrm", name="nrm_7")
            CPY = mybir.ActivationFunctionType.Copy
            for qc in range(4):
                for h in range(2):
                    col = qc * 128 + h * 64
                    if h == 0:
                        nc.scalar.activation(
                            nrm7[:, col:col + 64], O7[:, col:col + 64], CPY,
                            scale=rc7[:, qc * 2 + h: qc * 2 + h + 1])
                    else:
                        nc.vector.tensor_scalar_mul(
                            nrm7[:, col:col + 64], O7[:, col:col + 64],
                            rc7[:, qc * 2 + h: qc * 2 + h + 1])
                tp7 = pjp.tile([128, 128], BF, tag="pj", name=f"tp_7_{qc}")
                nc.tensor.transpose(
                    tp7[:, :], nrm7[:, qc * 128:(qc + 1) * 128], ident[:, :])
                nc.vector.tensor_copy(
                    onmT_sb[:, S + 3 * 512 + qc * 128: S + 3 * 512 +
                            (qc + 1) * 128], tp7[:, :])
                po2 = scp.tile([128, 1024], F32, tag="sc",
                               name=f"po3_{qc}")
                t = 12 + qc
                for n in range(2):
                    for hp in range(2):
                        nc.tensor.matmul(
                            po2[:, n * 512:(n + 1) * 512],
                            lhsT=onmT_sb[:, hp * S + 3 * 512 + qc * 128:
                                         hp * S + 3 * 512 + (qc + 1) * 128],
                            rhs=wo_sb[:, hp * D + n * 512:
                                      hp * D + (n + 1) * 512],
                            start=(hp == 0), stop=(hp == 1),
                        )
                for n in range(2):
                    ot = otp.tile([128, 512], BF, tag="ot",
                                  name=f"ot3_{qc}_{n}")
                    if (qc + n) % 2 == 0:
                        nc.scalar.copy(ot[:, :], po2[:, n * 512:(n + 1) * 512])
                    else:
                        nc.vector.tensor_copy(
                            ot[:, :], po2[:, n * 512:(n + 1) * 512])
                    nc.sync.dma_start(
                        out_d[t * 128:(t + 1) * 128, n * 512:(n + 1) * 512],
                        ot[:, :])
            while uidx < len(units):
                units[uidx][3]()
                uidx += 1

    nc.compile()
    return nc


def _get_nc(with_bias=False):
    if with_bias not in _nc_cache:
        _nc_cache[with_bias] = _build_bass(with_bias=with_bias)
    return _nc_cache[with_bias]


def _prepare_in_maps(x, wq, bq, wk, bk, wv, bv, wo):
    import ml_dtypes

    bf16 = ml_dtypes.bfloat16
    x = np.asarray(x, np.float32)
    wq, bq = np.asarray(wq, np.float32), np.asarray(bq, np.float32)
    wk, bk = np.asarray(wk, np.float32), np.asarray(bk, np.float32)
    wv, bv = np.asarray(wv, np.float32), np.asarray(bv, np.float32)
    wo = np.asarray(wo, np.float32)

    xT = [np.ascontiguousarray(x[b].T).astype(bf16) for b in range(B)]
    in_maps = []
    for c in range(NC):
        b, j = divmod(c, HPC)
        cs = slice(DHC * j, DHC * (j + 1))
        bias3 = np.concatenate([bq[cs], bk[cs], bv[cs]]).reshape(1, 3 * DHC).astype(bf16)
        in_maps.append(
            {
                "xT": xT[b],
                "wq_c": np.ascontiguousarray(wq[:, cs]).astype(bf16),
                "wk_c": np.ascontiguousarray(wk[:, cs]).astype(bf16),
                "wv_c": np.ascontiguousarray(wv[:, cs]).astype(bf16),
                "wo_c": np.ascontiguousarray(wo[cs, :]).astype(bf16),
                "bias3": np.ascontiguousarray(bias3),
            }
        )
    return in_maps


def _gather(parts, bo):
    bo = np.asarray(bo, np.float32)
    out = np.empty((B, S, D), np.float32)
    for b in range(B):
        acc = np.asarray(parts[HPC * b], np.float32)
        for j in range(1, HPC):
            acc = acc + np.asarray(parts[HPC * b + j], np.float32)
        out[b] = acc + bo
    return out


def kernel(x, wq, bq, wk, bk, wv, bv, wo, bo):
    from concourse import bass_utils

    in_maps = _prepare_in_maps(x, wq, bq, wk, bk, wv, bv, wo)
    with_bias = bool(
        np.any(np.asarray(bq)) or np.any(np.asarray(bk)) or np.any(np.asarray(bv))
    )
    res = bass_utils.run_bass_kernel_spmd(
        nc=_get_nc(with_bias), in_maps=in_maps, core_ids=list(range(NC))
    )
    parts = [np.asarray(r["out"], np.float32) for r in res.results]
    return _gather(parts, bo)


# revision 59
# speedup vs baseline: 1.0022x; 1.0022x over previous
"""Trainium2 Bass kernel for nn_Attention_80384607912675.

Multi-head attention (B=2, S=2048, D=1024, H=16, HD=64), fp32 reference.

Sharding (8 cores): data-parallel over batch (2) x tensor-parallel over heads
(4 head groups of 4 heads).  Core c handles batch c//4, heads [4*(c%4), 4*(c%4)+4).
wq/wk/wv split column-wise, wo split row-wise; the wo partial sums (and the
bias bo) are reduced on the host in fp32.

Per-core kernel (all matmuls bf16, fp32 PSUM accumulation):
  QT/KT = (x @ wq/k + b)^T   head-major [128 (2 heads x 64), 2048] per pair
  V     = x @ wv + bv        natural    [2048, 256] (xT as lhsT -> no transpose)
  per window w = (head pair hp, 512-wide q window qw), kp-chunk c:
    S^T[kp, (h, q)] = K_h^T (x) Q_h      packed [128, 1024] PSUM (A|B)
    P^T             = exp(S^T / 8)       one ACT instr -> bf16 SBUF
    O[q, (qc,h,hd)] += P^T(x)V chunks    [128 q, 64] tiles, full-partition PE
    rs[q, (qc,h)]   += P^T(x)1           rowsums, 1-wide matmuls
  drain: O+rs PSUM -> SBUF copy (frees the single O bank fast), reciprocal,
  per-partition normalize mul, PE transpose to O^T, out = O^T.T @ wo_c
  -> bf16 [2048, 1024] partial, DMA per [128, 512] tile.

Timing model notes (TimelineSim): PE matmul cost = out free size x 0.42ns x
k-chunks, independent of M/K utilization -- so PV uses full 128 q-partitions
(half the cost of the [65, 1024] O^T layout) and rowsums are near-free 1-wide
matmuls.  ACT exp is the second-closest engine to the roofline (~133us busy);
PE is the critical engine (~144us).  The schedule is exp-slot based: per slot
one sc QK pair + one exp, with PV lagged ~12 slots behind (the single PSUM O
bank serializes windows via the drain), and all projection/V/outproj work
packed greedily into per-slot PE budget via a FIFO unit queue with
earliest-slot gates and force-emit deadlines.  CRITICAL INVARIANT: a unit
producing data read at slot g must have deadline <= g-1, because slot g's
QK/exp are emitted first and emission order defines dependency direction in
the tile framework.  Dummy identity transposes burn the PE pstate ramp
(full speed needs ~3us of continuous execution) during the input-DMA wait;
xT streams in two s-halves so lead projections start ~5us in; window 7
drains directly from PSUM with normalize split ACT/DVE to shorten the tail.

Verified: CoreSim core-0 max err 1.7e-3 vs fp64 numpy; full 8-core test
rel err 5.3e-3 (gate 2e-2).  207.1us (baseline) -> 167.6us (-19%).
"""

import numpy as np

B, S, D, H = 2, 2048, 1024, 16
HD = D // H          # 64
HPC = 4              # heads per core
DHC = HPC * HD       # 256 head dims per core
KC = D // 128        # 8 contraction chunks
SB = S // 128        # 16 s blocks / kp chunks
NC = 8               # cores
NW = 8               # windows (2 head pairs x 4 q windows)

_nc_cache = {}


def _build_bass(with_bias=False):
    import concourse.mybir as mybir
    import concourse.tile as tile
    from concourse import bacc
    from concourse.masks import make_identity

    BF = mybir.dt.bfloat16
    F32 = mybir.dt.float32
    EXP = mybir.ActivationFunctionType.Exp

    nc = bacc.Bacc("TRN2")

    xT_d = nc.dram_tensor("xT", [D, S], BF, kind="ExternalInput")
    wq_d = nc.dram_tensor("wq_c", [D, DHC], BF, kind="ExternalInput")
    wk_d = nc.dram_tensor("wk_c", [D, DHC], BF, kind="ExternalInput")
    wv_d = nc.dram_tensor("wv_c", [D, DHC], BF, kind="ExternalInput")
    wo_d = nc.dram_tensor("wo_c", [DHC, D], BF, kind="ExternalInput")
    bias_d = nc.dram_tensor("bias3", [1, 3 * DHC], BF, kind="ExternalInput")
    out_d = nc.dram_tensor("out", [S, D], BF, kind="ExternalOutput")

    with tile.TileContext(nc) as tc:
        with (
            tc.tile_pool(name="persist", bufs=1) as pp,
            tc.tile_pool(name="sc", bufs=2, space="PSUM") as scp,
            tc.tile_pool(name="oacc", bufs=1, space="PSUM") as opp,
            tc.tile_pool(name="pj", bufs=2, space="PSUM") as pjp,
            tc.tile_pool(name="pt", bufs=12) as ptp,
            tc.tile_pool(name="osb", bufs=2) as osbp,
            tc.tile_pool(name="nrm", bufs=2) as nrmp,
            tc.tile_pool(name="rc", bufs=2) as rcp,
            tc.tile_pool(name="ot", bufs=8) as otp,
            tc.tile_pool(name="ot2", bufs=4) as ot2p,
        ):
            xT_sb = pp.tile([128, KC * S], BF, tag="xT", name="xT_sb")
            wq_sb = pp.tile([128, KC * DHC], BF, tag="wq", name="wq_sb")
            wk_sb = pp.tile([128, KC * DHC], BF, tag="wk", name="wk_sb")
            wv_sb = pp.tile([128, KC * DHC], BF, tag="wv", name="wv_sb")
            wo_sb = pp.tile([128, 2 * D], BF, tag="wo", name="wo_sb")
            qt_sb = pp.tile([128, 2 * S], BF, tag="qt", name="qt_sb")
            kt_sb = pp.tile([128, 2 * S], BF, tag="kt", name="kt_sb")
            v_sb = pp.tile([128, SB * DHC], BF, tag="v", name="v_sb")
            onmT_sb = pp.tile([128, 2 * S], BF, tag="onmT", name="onmT_sb")
            ident = pp.tile([128, 128], BF, tag="ident", name="ident")
            bias_sb = pp.tile([1, 3 * DHC], BF, tag="bias", name="bias_sb")
            ones16 = pp.tile([1, 512], BF, tag="ones16", name="ones16")
            ones_col = pp.tile([128, 1], BF, tag="ones_col", name="ones_col")

            # ---- input DMAs: small weights first, xT streamed in s-halves
            # (16 DMAs) so nt0/nt1-dependent projections start early; wo last.
            def load_w(w_sb, w_d):
                nc.sync.dma_start(
                    w_sb[:, :].rearrange("p (k d) -> p k d", d=DHC),
                    w_d[:, :].rearrange("(k p) d -> p k d", p=128),
                )

            def load_xt(k, h):
                nc.sync.dma_start(
                    xT_sb[:, k * S + h * 1024: k * S + (h + 1) * 1024],
                    xT_d[k * 128:(k + 1) * 128, h * 1024:(h + 1) * 1024],
                )

            load_w(wk_sb, wk_d)
            load_xt(0, 0)
            load_xt(1, 0)
            load_w(wq_sb, wq_d)
            for k in range(2, KC):
                load_xt(k, 0)
            load_w(wv_sb, wv_d)
            for k in range(KC):
                load_xt(k, 1)
            nc.sync.dma_start(bias_sb[:, :], bias_d[:, :])
            nc.sync.dma_start(
                wo_sb[:, :].rearrange("r (p d) -> r p d", d=D),
                wo_d[:, :].rearrange("(p r) d -> r p d", r=128),
            )
            nc.vector.memset(ones16[:, :], 1.0)
            nc.vector.memset(ones_col[:, :], 1.0)
            make_identity(nc, ident[:, :])

            bq = bias_sb[0:1, 0:DHC]
            bk = bias_sb[0:1, DHC:2 * DHC]
            bv = bias_sb[0:1, 2 * DHC:3 * DHC]

            # ---- Q/K projection tiles (p: head-pair block, nt: 512 s cols),
            # emitted in four 2-k-chunk quarters so no single filler slot
            # exceeds the exp budget (locally PE-stalled exp slots are never
            # recovered).
            pend = {}

            def qk_quarter(dst_sb, w_sb, bias, p, nt, q, eng="dve"):
                key = (dst_sb.tensor.name, p, nt)
                if q == 0:
                    ps = pjp.tile([128, 512], F32, tag="pj",
                                  name=f"qk_{key[0]}_{p}_{nt}")
                    pend[key] = ps
                else:
                    ps = pend[key]
                for k in (2 * q, 2 * q + 1):
                    nc.tensor.matmul(
                        ps[:, :],
                        lhsT=w_sb[:, k * DHC + p * 128: k * DHC + (p + 1) * 128],
                        rhs=xT_sb[:, k * S + nt * 512: k * S + (nt + 1) * 512],
                        start=(k == 0),
                        stop=(k == KC - 1 and not with_bias),
                    )
                if q == 3:
                    del pend[key]
                    if with_bias:
                        nc.tensor.matmul(
                            ps[:, :],
                            lhsT=bias[:, p * 128:(p + 1) * 128],
                            rhs=ones16[0:1, :],
                            start=False, stop=True,
                        )
                    dst = dst_sb[:, p * S + nt * 512: p * S + (nt + 1) * 512]
                    if eng == "act":
                        nc.scalar.copy(dst, ps[:, :])
                    else:
                        nc.vector.tensor_copy(dst, ps[:, :])

            # ---- V pair tiles: pair j covers s-chunks 2j, 2j+1 in natural
            # layout (lhsT = xT s-slice, rhs = wv chunk), in 4-matmul
            # quarters.  Pairs 0/1 run in the lead psum slots (lv).
            vpend = {}

            def v_bias_mms(ps):
                for h2 in range(2):
                    nc.tensor.matmul(
                        ps[:, h2 * 256:(h2 + 1) * 256],
                        lhsT=ones16[0:1, 0:128], rhs=bv[:, :],
                        start=False, stop=(h2 == 1),
                    )

            def v_quarter(j, q):
                if j < 2:
                    ps = lv[j]
                elif q == 0:
                    ps = pjp.tile([128, 512], F32, tag="pj", name=f"v_{j}")
                    vpend[j] = ps
                else:
                    ps = vpend[j]
                half = q // 2
                sc_ = 2 * j + half
                for k in range(4 * (q % 2), 4 * (q % 2) + 4):
                    nc.tensor.matmul(
                        ps[:, half * 256:(half + 1) * 256],
                        lhsT=xT_sb[:, k * S + sc_ * 128: k * S + (sc_ + 1) * 128],
                        rhs=wv_sb[:, k * DHC: k * DHC + DHC],
                        start=(k == 0 and half == 0),
                        stop=(k == KC - 1 and half == 1 and not with_bias),
                    )
                if q == 3:
                    vpend.pop(j, None)
                    if with_bias:
                        v_bias_mms(ps)
                    nc.vector.tensor_copy(
                        v_sb[:, 2 * j * DHC: (2 * j + 2) * DHC],
                        ps[:, 0:512])

            # ---- per-window attention pieces
            otiles = {}
            pts = {}

            def emit_qk_exp(w, c):
                hp, qw = w // 4, w % 4
                sc = scp.tile([128, 1024], F32, tag="sc", name=f"sc_{w}_{c}")
                for i in range(2):
                    nc.tensor.matmul(
                        sc[:, 512 * i:512 * (i + 1)],
                        lhsT=kt_sb[64 * i:64 * (i + 1),
                                   hp * S + c * 128: hp * S + (c + 1) * 128],
                        rhs=qt_sb[64 * i:64 * (i + 1),
                                  hp * S + qw * 512: hp * S + (qw + 1) * 512],
                        start=True, stop=True,
                    )
                pt = ptp.tile([128, 1024], BF, tag="pt", name=f"pt_{w}_{c}")
                nc.scalar.activation(pt[:, :], sc[:, :], EXP, scale=0.125)
                pts[(w, c)] = pt

            def pv(w, c):
                hp = w // 4
                if c == 0:
                    otiles[w] = opp.tile([128, 520], F32, tag="oacc",
                                         name=f"o_{w}")
                O = otiles[w]
                pt = pts.pop((w, c))
                for qc in range(4):
                    for h in range(2):
                        first = (c == 0 and qc == 0 and h == 0)
                        last = (c == SB - 1 and qc == 3 and h == 1)
                        lh = pt[:, h * 512 + qc * 128: h * 512 + (qc + 1) * 128]
                        nc.tensor.matmul(
                            O[:, qc * 128 + h * 64: qc * 128 + h * 64 + 64],
                            lhsT=lh,
                            rhs=v_sb[:, c * DHC + (2 * hp + h) * 64:
                                     c * DHC + (2 * hp + h) * 64 + 64],
                            start=first, stop=last,
                        )
                        nc.tensor.matmul(
                            O[:, 512 + qc * 2 + h: 513 + qc * 2 + h],
                            lhsT=lh, rhs=ones_col[:, 0:1],
                            start=first, stop=last,
                        )

            osbs = {}
            nrms = {}

            def drain_a(w):
                osb = osbp.tile([128, 520], F32, tag="osb", name=f"osb_{w}")
                nc.vector.tensor_copy(osb[:, :], otiles.pop(w)[:, :])
                osbs[w] = osb

            def drain_b(w):
                osb = osbs.pop(w)
                rc = rcp.tile([128, 8], F32, tag="rc", name=f"rc_{w}")
                nc.vector.reciprocal_approx_fast(
                    out=rc[:, :], in_=osb[:, 512:520])
                nrm = nrmp.tile([128, 512], BF, tag="nrm", name=f"nrm_{w}")
                for qc in range(4):
                    for h in range(2):
                        col = qc * 128 + h * 64
                        nc.vector.tensor_scalar_mul(
                            nrm[:, col:col + 64], osb[:, col:col + 64],
                            rc[:, qc * 2 + h: qc * 2 + h + 1])
                nrms[w] = nrm

            def drain_tp(w, qc):
                hp, qw = w // 4, w % 4
                nrm = nrms[w]
                tp = pjp.tile([128, 128], BF, tag="pj", name=f"tp_{w}_{qc}")
                nc.tensor.transpose(
                    tp[:, :], nrm[:, qc * 128:(qc + 1) * 128], ident[:, :])
                nc.vector.tensor_copy(
                    onmT_sb[:, hp * S + qw * 512 + qc * 128:
                            hp * S + qw * 512 + (qc + 1) * 128], tp[:, :])
                if qc == 3:
                    del nrms[w]

            def outproj(qw, qc, n, eng="dve"):
                t = qw * 4 + qc
                po = pjp.tile([128, 512], F32, tag="pj", name=f"po_{t}_{n}")
                for hp in range(2):
                    nc.tensor.matmul(
                        po[:, :],
                        lhsT=onmT_sb[:, hp * S + qw * 512 + qc * 128:
                                     hp * S + qw * 512 + (qc + 1) * 128],
                        rhs=wo_sb[:, hp * D + n * 512: hp * D + (n + 1) * 512],
                        start=(hp == 0), stop=(hp == 1),
                    )
                ot = otp.tile([128, 512], BF, tag="ot", name=f"ot_{t}_{n}")
                if eng == "act":
                    nc.scalar.copy(ot[:, :], po[:, :])
                else:
                    nc.vector.tensor_copy(ot[:, :], po[:, :])
                nc.sync.dma_start(
                    out_d[t * 128:(t + 1) * 128, n * 512:(n + 1) * 512],
                    ot[:, :])

            # ---- lead-in: kt/qt (p0, nt0) + V pairs 0,1 pipelined against
            # the arriving xT halves; kt/qt finish first so window 0 starts
            # as early as possible.  Dummy identity transposes keep the PE
            # continuously busy from t~0 so the pstate ramp (full speed after
            # 3us of uninterrupted execution) is burned during the input DMA
            # instead of doubling every lead matmul.
            def ramp(n):
                for _ in range(n):
                    nc.tensor.transpose(
                        dummy_bf[:, :], ident[:, :], ident[:, :])

            dummy_bf = pjp.tile([128, 128], BF, tag="pj", name="dummy_bf")
            lt = scp.tile([128, 1024], F32, tag="sc", name="lead_ktqt")
            lv = [opp.tile([128, 520], F32, tag="oacc", name="lead_v01"),
                  pjp.tile([128, 512], F32, tag="pj", name="lead_v23")]

            def lead_mm(k):
                for half, (w_sb,) in enumerate([(wk_sb,), (wq_sb,)]):
                    nc.tensor.matmul(
                        lt[:, half * 512:(half + 1) * 512],
                        lhsT=w_sb[:, k * DHC: k * DHC + 128],
                        rhs=xT_sb[:, k * S: k * S + 512],
                        start=(k == 0),
                        stop=(k == KC - 1 and not with_bias),
                    )

            def lead_vmm(k, pair):
                ps = lv[pair]
                for h2 in range(2):
                    sc_ = 2 * pair + h2
                    nc.tensor.matmul(
                        ps[:, h2 * 256:(h2 + 1) * 256],
                        lhsT=xT_sb[:, k * S + sc_ * 128: k * S + (sc_ + 1) * 128],
                        rhs=wv_sb[:, k * DHC: k * DHC + DHC],
                        start=(k == 0 and h2 == 0),
                        stop=(k == KC - 1 and h2 == 1 and not with_bias),
                    )

            ramp(15)
            for k in range(KC):
                lead_mm(k)
                if k < KC - 1:
                    ramp(2 if k < 4 else 5)
            if with_bias:
                for half, bias in enumerate([bk, bq]):
                    nc.tensor.matmul(
                        lt[:, half * 512:(half + 1) * 512],
                        lhsT=bias[:, 0:128], rhs=ones16[0:1, :],
                        start=False, stop=True,
                    )
            nc.scalar.copy(kt_sb[:, 0:512], lt[:, 0:512])
            nc.vector.tensor_copy(qt_sb[:, 0:512], lt[:, 512:1024])

            # ---- schedule ----
            # Mandatory per-slot items (PV cadence, drains, transposes) are
            # placed at fixed global slots g = 16*w + c.  Everything else
            # (projection/V quarters, outproj pieces) lives in a strict-FIFO
            # unit queue packed greedily against each slot's leftover PE
            # budget, with earliest-slot gates and force-emit deadlines.
            sched = {}
            mcost = {}

            def at(g, fn, cost=0):
                sched.setdefault(g, []).append(fn)
                mcost[g] = mcost.get(g, 0) + cost

            # PV cadence: PV(w, c) lags exp by ~12 slots; the single O bank
            # serializes windows (drain_a frees it).  Last two windows
            # compress so the tail stays short.
            for w in range(NW):
                for c in range(SB):
                    if w < 6:
                        g = 16 * w + 12 + c
                    elif w == 6:
                        g = 108 + c if c < 8 else 112 + (c - 8) // 2
                    else:
                        g = (120 + c if c < 4 else
                             124 + (c - 4) // 2 if c < 12 else 200)
                    at(g, lambda w=w, c=c: pv(w, c), 240)
            for w in range(NW - 1):
                if w < 6:
                    ga, gtp = 16 * w + 27, 16 * w + 30
                else:
                    ga, gtp = 115, 118
                at(ga, lambda w=w: drain_a(w))
                at(ga + 1, lambda w=w: drain_b(w))
                for qc in range(4):
                    at(gtp + qc // 2, lambda w=w, qc=qc: drain_tp(w, qc), 60)

            # filler unit queue
            units = []

            def unit(e, dls, cost, mk):
                for q, d in enumerate(dls):
                    units.append((e[q] if isinstance(e, list) else e, d, cost,
                                  mk(q)))

            def mk_qk(dst, wt, bias, p, nt, eng="dve"):
                return lambda q: (lambda: qk_quarter(dst, wt, bias, p, nt, q,
                                                     eng))

            def mk_v(j):
                return lambda q: (lambda: v_quarter(j, q))

            def mk_po(qw, qc, n, eng="dve"):
                return lambda q: (lambda: outproj(qw, qc, n, eng))

            unit(0, [0, 1, 2, 3], 430, mk_qk(kt_sb, wk_sb, bk, 0, 1, "act"))
            unit(1, [3, 4, 5, 6], 430, mk_v(0))
            unit([1, 2, 3, 5], [4, 5, 6, 7], 430,
                 mk_qk(kt_sb, wk_sb, bk, 0, 2, "act"))
            unit(1, [5, 6, 7, 8], 430, mk_v(1))
            unit(0, [8, 9, 10, 11], 430, mk_v(2))
            unit([1, 2, 3, 5], [8, 9, 10, 11], 430,
                 mk_qk(kt_sb, wk_sb, bk, 0, 3, "act"))
            unit(0, [10, 11, 12, 13], 430, mk_v(3))
            unit(0, [12, 13, 14, 15], 430, mk_qk(qt_sb, wq_sb, bq, 0, 1))
            unit([3, 5, 5, 5], [14, 15, 16, 17], 430, mk_v(4))
            unit([4, 5, 5, 6], [18, 19, 20, 21], 430, mk_v(5))
            unit([4, 5, 6, 6], [20, 21, 22, 23], 430, mk_v(6))
            unit([5, 6, 6, 7], [22, 23, 24, 25], 430, mk_v(7))
            unit(0, [28, 29, 30, 31], 430, mk_qk(qt_sb, wq_sb, bq, 0, 2))
            unit(0, [44, 45, 46, 47], 430, mk_qk(qt_sb, wq_sb, bq, 0, 3))
            unit(0, [60, 61, 62, 63], 430, mk_qk(kt_sb, wk_sb, bk, 1, 0))
            unit(0, [64, 65, 66, 67], 430, mk_qk(kt_sb, wk_sb, bk, 1, 1))
            unit(0, [68, 69, 70, 71], 430, mk_qk(kt_sb, wk_sb, bk, 1, 2))
            unit(0, [72, 73, 74, 75], 430, mk_qk(kt_sb, wk_sb, bk, 1, 3))
            unit(0, [60, 61, 62, 63], 430, mk_qk(qt_sb, wq_sb, bq, 1, 0))
            unit(0, [76, 77, 78, 79], 430, mk_qk(qt_sb, wq_sb, bq, 1, 1))
            unit([88, 89, 90, 91], [92, 93, 94, 95], 430, mk_qk(qt_sb, wq_sb, bq, 1, 2))
            for i in range(8):
                units.append((96, 118 + i, 430, (lambda i=i: outproj(
                    0, i // 2, i % 2))))
            unit([100, 101, 102, 103], [108, 109, 110, 111], 430, mk_qk(qt_sb, wq_sb, bq, 1, 3))
            for i in range(8):
                units.append((112, 113 + i, 430, (lambda i=i: outproj(
                    1, i // 2, i % 2))))
            for i in range(8):
                units.append((120, 120 + i, 430, (lambda i=i: outproj(
                    2, i // 2, i % 2))))

            # ---- main loop with budget packing
            uidx = 0
            for w in range(NW):
                for c in range(SB):
                    g = 16 * w + c
                    emit_qk_exp(w, c)
                    for fn in sched.get(g, ()):
                        fn()
                    budget = 1038 - 430 - mcost.get(g, 0)
                    spent = 0
                    while uidx < len(units):
                        e, d, cost, fn = units[uidx]
                        if e > g:
                            break
                        if (d <= g or spent + cost <= budget + 120
                                or (spent == 0 and budget >= 300)):
                            fn()
                            spent += cost
                            uidx += 1
                        else:
                            break

            # ---- tail: leftover units, then finish window 7.  onmT copies
            # (DVE) go ahead of the outproj staging copies; staging copies
            # alternate ACT/DVE.
            for fn in sched.get(200, ()):
                fn()
            # direct-from-PSUM drain of window 7: normalization runs on ACT
            # (idle after the last exp), per-qc so transposes and outproj
            # pipeline behind it.
            O7 = otiles.pop(7)
            rc7 = rcp.tile([128, 8], F32, tag="rc", name="rc_7")
            nc.vector.reciprocal_approx_fast(out=rc7[:, :], in_=O7[:, 512:520])
            nrm7 = nrmp.tile([128, 512], BF, tag="nrm", name="nrm_7")
            CPY = mybir.ActivationFunctionType.Copy
            for qc in range(4):
                for h in range(2):
                    col = qc * 128 + h * 64
                    if h == 0:
                        nc.scalar.activation(
                            nrm7[:, col:col + 64], O7[:, col:col + 64], CPY,
                            scale=rc7[:, qc * 2 + h: qc * 2 + h + 1])
                    else:
                        nc.vector.tensor_scalar_mul(
                            nrm7[:, col:col + 64], O7[:, col:col + 64],
                            rc7[:, qc * 2 + h: qc * 2 + h + 1])
                tp7 = pjp.tile([128, 128], BF, tag="pj", name=f"tp_7_{qc}")
                nc.tensor.transpose(
                    tp7[:, :], nrm7[:, qc * 128:(qc + 1) * 128], ident[:, :])
                nc.vector.tensor_copy(
                    onmT_sb[:, S + 3 * 512 + qc * 128: S + 3 * 512 +
                            (qc + 1) * 128], tp7[:, :])
                po2 = scp.tile([128, 1024], F32, tag="sc",
                               name=f"po3_{qc}")
                t = 12 + qc
                for n in range(2):
                    for hp in range(2):
                        nc.tensor.matmul(
                            po2[:, n * 512:(n + 1) * 512],
                            lhsT=onmT_sb[:, hp * S + 3 * 512 + qc * 128:
                                         hp * S + 3 * 512 + (qc + 1) * 128],
                            rhs=wo_sb[:, hp * D + n * 512:
                                      hp * D + (n + 1) * 512],
                            start=(hp == 0), stop=(hp == 1),
                        )
                ot = ot2p.tile([128, 1024], BF, tag="ot2",
                               name=f"ot3_{qc}")
                if qc % 2 == 0:
                    nc.scalar.copy(ot[:, :], po2[:, :])
                else:
                    nc.vector.tensor_copy(ot[:, :], po2[:, :])
                nc.sync.dma_start(out_d[t * 128:(t + 1) * 128, :], ot[:, :])
            while uidx < len(units):
                units[uidx][3]()
                uidx += 1

    nc.compile()
    return nc


def _get_nc(with_bias=False):
    if with_bias not in _nc_cache:
        _nc_cache[with_bias] = _build_bass(with_bias=with_bias)
    return _nc_cache[with_bias]


def _prepare_in_maps(x, wq, bq, wk, bk, wv, bv, wo):
    import ml_dtypes

    bf16 = ml_dtypes.bfloat16
    x = np.asarray(x, np.float32)
    wq, bq = np.asarray(wq, np.float32), np.asarray(bq, np.float32)
    wk, bk = np.asarray(wk, np.float32), np.asarray(bk, np.float32)
    wv, bv = np.asarray(wv, np.float32), np.asarray(bv, np.float32)
    wo = np.asarray(wo, np.float32)

    xT = [np.ascontiguousarray(x[b].T).astype(bf16) for b in range(B)]
    in_maps = []
    for c in range(NC):
        b, j = divmod(c, HPC)
        cs = slice(DHC * j, DHC * (j + 1))
        bias3 = np.concatenate([bq[cs], bk[cs], bv[cs]]).reshape(1, 3 * DHC).astype(bf16)
        in_maps.append(
            {
                "xT": xT[b],
                "wq_c": np.ascontiguousarray(wq[:, cs]).astype(bf16),
                "wk_c": np.ascontiguousarray(wk[:, cs]).astype(bf16),
                "wv_c": np.ascontiguousarray(wv[:, cs]).astype(bf16),
                "wo_c": np.ascontiguousarray(wo[cs, :]).astype(bf16),
                "bias3": np.ascontiguousarray(bias3),
            }
        )
    return in_maps


def _gather(parts, bo):
    bo = np.asarray(bo, np.float32)
    out = np.empty((B, S, D), np.float32)
    for b in range(B):
        acc = np.asarray(parts[HPC * b], np.float32)
        for j in range(1, HPC):
            acc = acc + np.asarray(parts[HPC * b + j], np.float32)
        out[b] = acc + bo
    return out


def kernel(x, wq, bq, wk, bk, wv, bv, wo, bo):
    from concourse import bass_utils

    in_maps = _prepare_in_maps(x, wq, bq, wk, bk, wv, bv, wo)
    with_bias = bool(
        np.any(np.asarray(bq)) or np.any(np.asarray(bk)) or np.any(np.asarray(bv))
    )
    res = bass_utils.run_bass_kernel_spmd(
        nc=_get_nc(with_bias), in_maps=in_maps, core_ids=list(range(NC))
    )
    parts = [np.asarray(r["out"], np.float32) for r in res.results]
    return _gather(parts, bo)


# revision 60
# speedup vs baseline: 1.0023x; 1.0001x over previous
"""Trainium2 Bass kernel for nn_Attention_80384607912675.

Multi-head attention (B=2, S=2048, D=1024, H=16, HD=64), fp32 reference.

Sharding (8 cores): data-parallel over batch (2) x tensor-parallel over heads
(4 head groups of 4 heads).  Core c handles batch c//4, heads [4*(c%4), 4*(c%4)+4).
wq/wk/wv split column-wise, wo split row-wise; the wo partial sums (and the
bias bo) are reduced on the host in fp32.

Per-core kernel (all matmuls bf16, fp32 PSUM accumulation):
  QT/KT = (x @ wq/k + b)^T   head-major [128 (2 heads x 64), 2048] per pair
  V     = x @ wv + bv        natural    [2048, 256] (xT as lhsT -> no transpose)
  per window w = (head pair hp, 512-wide q window qw), kp-chunk c:
    S^T[kp, (h, q)] = K_h^T (x) Q_h      packed [128, 1024] PSUM (A|B)
    P^T             = exp(S^T / 8)       one ACT instr -> bf16 SBUF
    O[q, (qc,h,hd)] += P^T(x)V chunks    [128 q, 64] tiles, full-partition PE
    rs[q, (qc,h)]   += P^T(x)1           rowsums, 1-wide matmuls
  drain: O+rs PSUM -> SBUF copy (frees the single O bank fast), reciprocal,
  per-partition normalize mul, PE transpose to O^T, out = O^T.T @ wo_c
  -> bf16 [2048, 1024] partial, DMA per [128, 512] tile.

Timing model notes (TimelineSim): PE matmul cost = out free size x 0.42ns x
k-chunks, independent of M/K utilization -- so PV uses full 128 q-partitions
(half the cost of the [65, 1024] O^T layout) and rowsums are near-free 1-wide
matmuls.  ACT exp is the second-closest engine to the roofline (~133us busy);
PE is the critical engine (~144us).  The schedule is exp-slot based: per slot
one sc QK pair + one exp, with PV lagged ~12 slots behind (the single PSUM O
bank serializes windows via the drain), and all projection/V/outproj work
packed greedily into per-slot PE budget via a FIFO unit queue with
earliest-slot gates and force-emit deadlines.  CRITICAL INVARIANT: a unit
producing data read at slot g must have deadline <= g-1, because slot g's
QK/exp are emitted first and emission order defines dependency direction in
the tile framework.  Dummy identity transposes burn the PE pstate ramp
(full speed needs ~3us of continuous execution) during the input-DMA wait;
xT streams in two s-halves so lead projections start ~5us in; window 7
drains directly from PSUM with normalize split ACT/DVE to shorten the tail.

Verified: CoreSim core-0 max err 1.7e-3 vs fp64 numpy; full 8-core test
rel err 5.3e-3 (gate 2e-2).  207.1us (baseline) -> 167.6us (-19%).
"""

import numpy as np

B, S, D, H = 2, 2048, 1024, 16
HD = D // H          # 64
HPC = 4              # heads per core
DHC = HPC * HD       # 256 head dims per core
KC = D // 128        # 8 contraction chunks
SB = S // 128        # 16 s blocks / kp chunks
NC = 8               # cores
NW = 8               # windows (2 head pairs x 4 q windows)

_nc_cache = {}


def _build_bass(with_bias=False):
    import concourse.mybir as mybir
    import concourse.tile as tile
    from concourse import bacc
    from concourse.masks import make_identity

    BF = mybir.dt.bfloat16
    F32 = mybir.dt.float32
    EXP = mybir.ActivationFunctionType.Exp

    nc = bacc.Bacc("TRN2")

    xT_d = nc.dram_tensor("xT", [D, S], BF, kind="ExternalInput")
    wq_d = nc.dram_tensor("wq_c", [D, DHC], BF, kind="ExternalInput")
    wk_d = nc.dram_tensor("wk_c", [D, DHC], BF, kind="ExternalInput")
    wv_d = nc.dram_tensor("wv_c", [D, DHC], BF, kind="ExternalInput")
    wo_d = nc.dram_tensor("wo_c", [DHC, D], BF, kind="ExternalInput")
    bias_d = nc.dram_tensor("bias3", [1, 3 * DHC], BF, kind="ExternalInput")
    out_d = nc.dram_tensor("out", [S, D], BF, kind="ExternalOutput")

    with tile.TileContext(nc) as tc:
        with (
            tc.tile_pool(name="persist", bufs=1) as pp,
            tc.tile_pool(name="sc", bufs=2, space="PSUM") as scp,
            tc.tile_pool(name="oacc", bufs=1, space="PSUM") as opp,
            tc.tile_pool(name="pj", bufs=2, space="PSUM") as pjp,
            tc.tile_pool(name="pt", bufs=12) as ptp,
            tc.tile_pool(name="osb", bufs=2) as osbp,
            tc.tile_pool(name="nrm", bufs=2) as nrmp,
            tc.tile_pool(name="rc", bufs=2) as rcp,
            tc.tile_pool(name="ot", bufs=8) as otp,
            tc.tile_pool(name="ot2", bufs=4) as ot2p,
        ):
            xT_sb = pp.tile([128, KC * S], BF, tag="xT", name="xT_sb")
            wq_sb = pp.tile([128, KC * DHC], BF, tag="wq", name="wq_sb")
            wk_sb = pp.tile([128, KC * DHC], BF, tag="wk", name="wk_sb")
            wv_sb = pp.tile([128, KC * DHC], BF, tag="wv", name="wv_sb")
            wo_sb = pp.tile([128, 2 * D], BF, tag="wo", name="wo_sb")
            qt_sb = pp.tile([128, 2 * S], BF, tag="qt", name="qt_sb")
            kt_sb = pp.tile([128, 2 * S], BF, tag="kt", name="kt_sb")
            v_sb = pp.tile([128, SB * DHC], BF, tag="v", name="v_sb")
            onmT_sb = pp.tile([128, 2 * S], BF, tag="onmT", name="onmT_sb")
            ident = pp.tile([128, 128], BF, tag="ident", name="ident")
            bias_sb = pp.tile([1, 3 * DHC], BF, tag="bias", name="bias_sb")
            ones16 = pp.tile([1, 512], BF, tag="ones16", name="ones16")
            ones_col = pp.tile([128, 1], BF, tag="ones_col", name="ones_col")

            # ---- input DMAs: small weights first, xT streamed in s-halves
            # (16 DMAs) so nt0/nt1-dependent projections start early; wo last.
            def load_w(w_sb, w_d):
                nc.sync.dma_start(
                    w_sb[:, :].rearrange("p (k d) -> p k d", d=DHC),
                    w_d[:, :].rearrange("(k p) d -> p k d", p=128),
                )

            def load_xt(k, h):
                nc.sync.dma_start(
                    xT_sb[:, k * S + h * 1024: k * S + (h + 1) * 1024],
                    xT_d[k * 128:(k + 1) * 128, h * 1024:(h + 1) * 1024],
                )

            load_w(wk_sb, wk_d)
            load_xt(0, 0)
            load_xt(1, 0)
            load_w(wq_sb, wq_d)
            for k in range(2, KC):
                load_xt(k, 0)
            load_w(wv_sb, wv_d)
            for k in range(KC):
                load_xt(k, 1)
            nc.sync.dma_start(bias_sb[:, :], bias_d[:, :])
            nc.sync.dma_start(
                wo_sb[:, :].rearrange("r (p d) -> r p d", d=D),
                wo_d[:, :].rearrange("(p r) d -> r p d", r=128),
            )
            nc.vector.memset(ones16[:, :], 1.0)
            nc.vector.memset(ones_col[:, :], 1.0)
            make_identity(nc, ident[:, :])

            bq = bias_sb[0:1, 0:DHC]
            bk = bias_sb[0:1, DHC:2 * DHC]
            bv = bias_sb[0:1, 2 * DHC:3 * DHC]

            # ---- Q/K projection tiles (p: head-pair block, nt: 512 s cols),
            # emitted in four 2-k-chunk quarters so no single filler slot
            # exceeds the exp budget (locally PE-stalled exp slots are never
            # recovered).
            pend = {}

            def qk_quarter(dst_sb, w_sb, bias, p, nt, q, eng="dve"):
                key = (dst_sb.tensor.name, p, nt)
                if q == 0:
                    ps = pjp.tile([128, 512], F32, tag="pj",
                                  name=f"qk_{key[0]}_{p}_{nt}")
                    pend[key] = ps
                else:
                    ps = pend[key]
                for k in (2 * q, 2 * q + 1):
                    nc.tensor.matmul(
                        ps[:, :],
                        lhsT=w_sb[:, k * DHC + p * 128: k * DHC + (p + 1) * 128],
                        rhs=xT_sb[:, k * S + nt * 512: k * S + (nt + 1) * 512],
                        start=(k == 0),
                        stop=(k == KC - 1 and not with_bias),
                    )
                if q == 3:
                    del pend[key]
                    if with_bias:
                        nc.tensor.matmul(
                            ps[:, :],
                            lhsT=bias[:, p * 128:(p + 1) * 128],
                            rhs=ones16[0:1, :],
                            start=False, stop=True,
                        )
                    dst = dst_sb[:, p * S + nt * 512: p * S + (nt + 1) * 512]
                    if eng == "act":
                        nc.scalar.copy(dst, ps[:, :])
                    else:
                        nc.vector.tensor_copy(dst, ps[:, :])

            # ---- V pair tiles: pair j covers s-chunks 2j, 2j+1 in natural
            # layout (lhsT = xT s-slice, rhs = wv chunk), in 4-matmul
            # quarters.  Pairs 0/1 run in the lead psum slots (lv).
            vpend = {}

            def v_bias_mms(ps):
                for h2 in range(2):
                    nc.tensor.matmul(
                        ps[:, h2 * 256:(h2 + 1) * 256],
                        lhsT=ones16[0:1, 0:128], rhs=bv[:, :],
                        start=False, stop=(h2 == 1),
                    )

            def v_quarter(j, q):
                if j < 2:
                    ps = lv[j]
                elif q == 0:
                    ps = pjp.tile([128, 512], F32, tag="pj", name=f"v_{j}")
                    vpend[j] = ps
                else:
                    ps = vpend[j]
                half = q // 2
                sc_ = 2 * j + half
                for k in range(4 * (q % 2), 4 * (q % 2) + 4):
                    nc.tensor.matmul(
                        ps[:, half * 256:(half + 1) * 256],
                        lhsT=xT_sb[:, k * S + sc_ * 128: k * S + (sc_ + 1) * 128],
                        rhs=wv_sb[:, k * DHC: k * DHC + DHC],
                        start=(k == 0 and half == 0),
                        stop=(k == KC - 1 and half == 1 and not with_bias),
                    )
                if q == 3:
                    vpend.pop(j, None)
                    if with_bias:
                        v_bias_mms(ps)
                    nc.vector.tensor_copy(
                        v_sb[:, 2 * j * DHC: (2 * j + 2) * DHC],
                        ps[:, 0:512])

            # ---- per-window attention pieces
            otiles = {}
            pts = {}

            def emit_qk_exp(w, c):
                hp, qw = w // 4, w % 4
                sc = scp.tile([128, 1024], F32, tag="sc", name=f"sc_{w}_{c}")
                for i in range(2):
                    nc.tensor.matmul(
                        sc[:, 512 * i:512 * (i + 1)],
                        lhsT=kt_sb[64 * i:64 * (i + 1),
                                   hp * S + c * 128: hp * S + (c + 1) * 128],
                        rhs=qt_sb[64 * i:64 * (i + 1),
                                  hp * S + qw * 512: hp * S + (qw + 1) * 512],
                        start=True, stop=True,
                    )
                pt = ptp.tile([128, 1024], BF, tag="pt", name=f"pt_{w}_{c}")
                nc.scalar.activation(pt[:, :], sc[:, :], EXP, scale=0.125)
                pts[(w, c)] = pt

            def pv(w, c):
                hp = w // 4
                if c == 0:
                    otiles[w] = opp.tile([128, 520], F32, tag="oacc",
                                         name=f"o_{w}")
                O = otiles[w]
                pt = pts.pop((w, c))
                for qc in range(4):
                    for h in range(2):
                        first = (c == 0 and qc == 0 and h == 0)
                        last = (c == SB - 1 and qc == 3 and h == 1)
                        lh = pt[:, h * 512 + qc * 128: h * 512 + (qc + 1) * 128]
                        nc.tensor.matmul(
                            O[:, qc * 128 + h * 64: qc * 128 + h * 64 + 64],
                            lhsT=lh,
                            rhs=v_sb[:, c * DHC + (2 * hp + h) * 64:
                                     c * DHC + (2 * hp + h) * 64 + 64],
                            start=first, stop=last,
                        )
                        nc.tensor.matmul(
                            O[:, 512 + qc * 2 + h: 513 + qc * 2 + h],
                            lhsT=lh, rhs=ones_col[:, 0:1],
                            start=first, stop=last,
                        )

            osbs = {}
            nrms = {}

            def drain_a(w):
                osb = osbp.tile([128, 520], F32, tag="osb", name=f"osb_{w}")
                nc.vector.tensor_copy(osb[:, :], otiles.pop(w)[:, :])
                osbs[w] = osb

            def drain_b(w):
                osb = osbs.pop(w)
                rc = rcp.tile([128, 8], F32, tag="rc", name=f"rc_{w}")
                nc.vector.reciprocal_approx_fast(
                    out=rc[:, :], in_=osb[:, 512:520])
                nrm = nrmp.tile([128, 512], BF, tag="nrm", name=f"nrm_{w}")
                for qc in range(4):
                    for h in range(2):
                        col = qc * 128 + h * 64
                        nc.vector.tensor_scalar_mul(
                            nrm[:, col:col + 64], osb[:, col:col + 64],
                            rc[:, qc * 2 + h: qc * 2 + h + 1])
                nrms[w] = nrm

            def drain_tp(w, qc):
                hp, qw = w // 4, w % 4
                nrm = nrms[w]
                tp = pjp.tile([128, 128], BF, tag="pj", name=f"tp_{w}_{qc}")
                nc.tensor.transpose(
                    tp[:, :], nrm[:, qc * 128:(qc + 1) * 128], ident[:, :])
                nc.vector.tensor_copy(
                    onmT_sb[:, hp * S + qw * 512 + qc * 128:
                            hp * S + qw * 512 + (qc + 1) * 128], tp[:, :])
                if qc == 3:
                    del nrms[w]

            def outproj(qw, qc, n, eng="dve"):
                t = qw * 4 + qc
                po = pjp.tile([128, 512], F32, tag="pj", name=f"po_{t}_{n}")
                for hp in range(2):
                    nc.tensor.matmul(
                        po[:, :],
                        lhsT=onmT_sb[:, hp * S + qw * 512 + qc * 128:
                                     hp * S + qw * 512 + (qc + 1) * 128],
                        rhs=wo_sb[:, hp * D + n * 512: hp * D + (n + 1) * 512],
                        start=(hp == 0), stop=(hp == 1),
                    )
                ot = otp.tile([128, 512], BF, tag="ot", name=f"ot_{t}_{n}")
                if eng == "act":
                    nc.scalar.copy(ot[:, :], po[:, :])
                else:
                    nc.vector.tensor_copy(ot[:, :], po[:, :])
                nc.sync.dma_start(
                    out_d[t * 128:(t + 1) * 128, n * 512:(n + 1) * 512],
                    ot[:, :])

            # ---- lead-in: kt/qt (p0, nt0) + V pairs 0,1 pipelined against
            # the arriving xT halves; kt/qt finish first so window 0 starts
            # as early as possible.  Dummy identity transposes keep the PE
            # continuously busy from t~0 so the pstate ramp (full speed after
            # 3us of uninterrupted execution) is burned during the input DMA
            # instead of doubling every lead matmul.
            def ramp(n):
                for _ in range(n):
                    nc.tensor.transpose(
                        dummy_bf[:, :], ident[:, :], ident[:, :])

            dummy_bf = pjp.tile([128, 128], BF, tag="pj", name="dummy_bf")
            lt = scp.tile([128, 1024], F32, tag="sc", name="lead_ktqt")
            lv = [opp.tile([128, 520], F32, tag="oacc", name="lead_v01"),
                  pjp.tile([128, 512], F32, tag="pj", name="lead_v23")]

            def lead_mm(k):
                for half, (w_sb,) in enumerate([(wk_sb,), (wq_sb,)]):
                    nc.tensor.matmul(
                        lt[:, half * 512:(half + 1) * 512],
                        lhsT=w_sb[:, k * DHC: k * DHC + 128],
                        rhs=xT_sb[:, k * S: k * S + 512],
                        start=(k == 0),
                        stop=(k == KC - 1 and not with_bias),
                    )

            def lead_vmm(k, pair):
                ps = lv[pair]
                for h2 in range(2):
                    sc_ = 2 * pair + h2
                    nc.tensor.matmul(
                        ps[:, h2 * 256:(h2 + 1) * 256],
                        lhsT=xT_sb[:, k * S + sc_ * 128: k * S + (sc_ + 1) * 128],
                        rhs=wv_sb[:, k * DHC: k * DHC + DHC],
                        start=(k == 0 and h2 == 0),
                        stop=(k == KC - 1 and h2 == 1 and not with_bias),
                    )

            ramp(15)
            for k in range(KC):
                lead_mm(k)
                if k < KC - 1:
                    ramp(2 if k < 4 else 5)
            if with_bias:
                for half, bias in enumerate([bk, bq]):
                    nc.tensor.matmul(
                        lt[:, half * 512:(half + 1) * 512],
                        lhsT=bias[:, 0:128], rhs=ones16[0:1, :],
                        start=False, stop=True,
                    )
            nc.scalar.copy(kt_sb[:, 0:512], lt[:, 0:512])
            nc.vector.tensor_copy(qt_sb[:, 0:512], lt[:, 512:1024])

            # ---- schedule ----
            # Mandatory per-slot items (PV cadence, drains, transposes) are
            # placed at fixed global slots g = 16*w + c.  Everything else
            # (projection/V quarters, outproj pieces) lives in a strict-FIFO
            # unit queue packed greedily against each slot's leftover PE
            # budget, with earliest-slot gates and force-emit deadlines.
            sched = {}
            mcost = {}

            def at(g, fn, cost=0):
                sched.setdefault(g, []).append(fn)
                mcost[g] = mcost.get(g, 0) + cost

            # PV cadence: PV(w, c) lags exp by ~12 slots; the single O bank
            # serializes windows (drain_a frees it).  Last two windows
            # compress so the tail stays short.
            for w in range(NW):
                for c in range(SB):
                    if w < 6:
                        g = 16 * w + 12 + c
                    elif w == 6:
                        g = 108 + c if c < 8 else 112 + (c - 8) // 2
                    else:
                        g = (120 + c if c < 4 else
                             124 + (c - 4) // 2 if c < 12 else 200)
                    at(g, lambda w=w, c=c: pv(w, c), 240)
            for w in range(NW - 1):
                if w < 6:
                    ga, gtp = 16 * w + 27, 16 * w + 30
                else:
                    ga, gtp = 115, 118
                at(ga, lambda w=w: drain_a(w))
                at(ga + 1, lambda w=w: drain_b(w))
                for qc in range(4):
                    at(gtp + qc // 2, lambda w=w, qc=qc: drain_tp(w, qc), 60)

            # filler unit queue
            units = []

            def unit(e, dls, cost, mk):
                for q, d in enumerate(dls):
                    units.append((e[q] if isinstance(e, list) else e, d, cost,
                                  mk(q)))

            def mk_qk(dst, wt, bias, p, nt, eng="dve"):
                return lambda q: (lambda: qk_quarter(dst, wt, bias, p, nt, q,
                                                     eng))

            def mk_v(j):
                return lambda q: (lambda: v_quarter(j, q))

            def mk_po(qw, qc, n, eng="dve"):
                return lambda q: (lambda: outproj(qw, qc, n, eng))

            unit(0, [0, 1, 2, 3], 430, mk_qk(kt_sb, wk_sb, bk, 0, 1, "act"))
            unit(1, [3, 4, 5, 6], 430, mk_v(0))
            unit([1, 2, 3, 5], [4, 5, 6, 7], 430,
                 mk_qk(kt_sb, wk_sb, bk, 0, 2, "act"))
            unit(1, [5, 6, 7, 8], 430, mk_v(1))
            unit(0, [8, 9, 10, 11], 430, mk_v(2))
            unit([1, 2, 3, 5], [8, 9, 10, 11], 430,
                 mk_qk(kt_sb, wk_sb, bk, 0, 3, "act"))
            unit(0, [10, 11, 12, 13], 430, mk_v(3))
            unit(0, [12, 13, 14, 15], 430, mk_qk(qt_sb, wq_sb, bq, 0, 1))
            unit([3, 5, 5, 5], [14, 15, 16, 17], 430, mk_v(4))
            unit([4, 5, 5, 6], [18, 19, 20, 21], 430, mk_v(5))
            unit([4, 5, 6, 6], [20, 21, 22, 23], 430, mk_v(6))
            unit([5, 6, 6, 7], [22, 23, 24, 25], 430, mk_v(7))
            unit(0, [28, 29, 30, 31], 430, mk_qk(qt_sb, wq_sb, bq, 0, 2))
            unit(0, [44, 45, 46, 47], 430, mk_qk(qt_sb, wq_sb, bq, 0, 3))
            unit(0, [60, 61, 62, 63], 430, mk_qk(kt_sb, wk_sb, bk, 1, 0))
            unit(0, [64, 65, 66, 67], 430, mk_qk(kt_sb, wk_sb, bk, 1, 1))
            unit(0, [68, 69, 70, 71], 430, mk_qk(kt_sb, wk_sb, bk, 1, 2))
            unit(0, [72, 73, 74, 75], 430, mk_qk(kt_sb, wk_sb, bk, 1, 3))
            unit(0, [60, 61, 62, 63], 430, mk_qk(qt_sb, wq_sb, bq, 1, 0))
            unit(0, [76, 77, 78, 79], 430, mk_qk(qt_sb, wq_sb, bq, 1, 1))
            unit([88, 89, 90, 91], [92, 93, 94, 95], 430, mk_qk(qt_sb, wq_sb, bq, 1, 2))
            for i in range(8):
                units.append((96, 118 + i, 430, (lambda i=i: outproj(
                    0, i // 2, i % 2))))
            unit([100, 101, 102, 103], [108, 109, 110, 111], 430, mk_qk(qt_sb, wq_sb, bq, 1, 3))
            for i in range(8):
                units.append((112, 113 + i, 430, (lambda i=i: outproj(
                    1, i // 2, i % 2))))
            for i in range(8):
                units.append((120, 120 + i, 430, (lambda i=i: outproj(
                    2, i // 2, i % 2))))

            # ---- main loop with budget packing
            uidx = 0
            for w in range(NW):
                for c in range(SB):
                    g = 16 * w + c
                    emit_qk_exp(w, c)
                    for fn in sched.get(g, ()):
                        fn()
                    budget = 1038 - 430 - mcost.get(g, 0)
                    spent = 0
                    while uidx < len(units):
                        e, d, cost, fn = units[uidx]
                        if e > g:
                            break
                        if (d <= g or spent + cost <= budget + 120
                                or (spent == 0 and budget >= 300)):
                            fn()
                            spent += cost
                            uidx += 1
                        else:
                            break

            # ---- tail: leftover units, then finish window 7.  onmT copies
            # (DVE) go ahead of the outproj staging copies; staging copies
            # alternate ACT/DVE.
            for fn in sched.get(200, ()):
                fn()
            # direct-from-PSUM drain of window 7: normalization runs on ACT
            # (idle after the last exp), per-qc so transposes and outproj
            # pipeline behind it.
            O7 = otiles.pop(7)
            rc7 = rcp.tile([128, 8], F32, tag="rc", name="rc_7")
            nc.vector.reciprocal_approx_fast(out=rc7[:, :], in_=O7[:, 512:520])
            nrm7 = nrmp.tile([128, 512], BF, tag="nrm", name="nrm_7")
            CPY = mybir.ActivationFunctionType.Copy
            for qc in range(4):
                for h in range(2):
                    col = qc * 128 + h * 64
                    if h == 0:
                        nc.scalar.activation(
                            nrm7[:, col:col + 64], O7[:, col:col + 64], CPY,
                            scale=rc7[:, qc * 2 + h: qc * 2 + h + 1])
                    else:
                        nc.vector.tensor_scalar_mul(
                            nrm7[:, col:col + 64], O7[:, col:col + 64],
                            rc7[:, qc * 2 + h: qc * 2 + h + 1])
                tp7 = pjp.tile([128, 128], BF, tag="pj", name=f"tp_7_{qc}")
                nc.tensor.transpose(
                    tp7[:, :], nrm7[:, qc * 128:(qc + 1) * 128], ident[:, :])
                nc.vector.tensor_copy(
                    onmT_sb[:, S + 3 * 512 + qc * 128: S + 3 * 512 +
                            (qc + 1) * 128], tp7[:, :])
                po2 = scp.tile([128, 1024], F32, tag="sc",
                               name=f"po3_{qc}")
                t = 12 + qc
                for n in range(2):
                    for hp in range(2):
                        nc.tensor.matmul(
                            po2[:, n * 512:(n + 1) * 512],
                            lhsT=onmT_sb[:, hp * S + 3 * 512 + qc * 128:
                                         hp * S + 3 * 512 + (qc + 1) * 128],
                            rhs=wo_sb[:, hp * D + n * 512:
                                      hp * D + (n + 1) * 512],
                            start=(hp == 0), stop=(hp == 1),
                        )
                ot = ot2p.tile([128, 1024], BF, tag="ot2",
                               name=f"ot3_{qc}")
                if qc == 3:
                    # split the last tile across both engines so the final
                    # DMA launches as early as possible
                    nc.scalar.copy(ot[:, 0:512], po2[:, 0:512])
                    nc.vector.tensor_copy(ot[:, 512:1024], po2[:, 512:1024])
                    nc.sync.dma_start(
                        out_d[t * 128:(t + 1) * 128, 0:512], ot[:, 0:512])
                    nc.sync.dma_start(
                        out_d[t * 128:(t + 1) * 128, 512:1024],
                        ot[:, 512:1024])
                else:
                    if qc % 2 == 0:
                        nc.scalar.copy(ot[:, :], po2[:, :])
                    else:
                        nc.vector.tensor_copy(ot[:, :], po2[:, :])
                    nc.sync.dma_start(
                        out_d[t * 128:(t + 1) * 128, :], ot[:, :])
            while uidx < len(units):
                units[uidx][3]()
                uidx += 1

    nc.compile()
    return nc


def _get_nc(with_bias=False):
    if with_bias not in _nc_cache:
        _nc_cache[with_bias] = _build_bass(with_bias=with_bias)
    return _nc_cache[with_bias]


def _prepare_in_maps(x, wq, bq, wk, bk, wv, bv, wo):
    import ml_dtypes

    bf16 = ml_dtypes.bfloat16
    x = np.asarray(x, np.float32)
    wq, bq = np.asarray(wq, np.float32), np.asarray(bq, np.float32)
    wk, bk = np.asarray(wk, np.float32), np.asarray(bk, np.float32)
    wv, bv = np.asarray(wv, np.float32), np.asarray(bv, np.float32)
    wo = np.asarray(wo, np.float32)

    xT = [np.ascontiguousarray(x[b].T).astype(bf16) for b in range(B)]
    in_maps = []
    for c in range(NC):
        b, j = divmod(c, HPC)
        cs = slice(DHC * j, DHC * (j + 1))
        bias3 = np.concatenate([bq[cs], bk[cs], bv[cs]]).reshape(1, 3 * DHC).astype(bf16)
        in_maps.append(
            {
                "xT": xT[b],
                "wq_c": np.ascontiguousarray(wq[:, cs]).astype(bf16),
                "wk_c": np.ascontiguousarray(wk[:, cs]).astype(bf16),
                "wv_c": np.ascontiguousarray(wv[:, cs]).astype(bf16),
                "wo_c": np.ascontiguousarray(wo[cs, :]).astype(bf16),
                "bias3": np.ascontiguousarray(bias3),
            }
        )
    return in_maps


def _gather(parts, bo):
    bo = np.asarray(bo, np.float32)
    out = np.empty((B, S, D), np.float32)
    for b in range(B):
        acc = np.asarray(parts[HPC * b], np.float32)
        for j in range(1, HPC):
            acc = acc + np.asarray(parts[HPC * b + j], np.float32)
        out[b] = acc + bo
    return out


def kernel(x, wq, bq, wk, bk, wv, bv, wo, bo):
    from concourse import bass_utils

    in_maps = _prepare_in_maps(x, wq, bq, wk, bk, wv, bv, wo)
    with_bias = bool(
        np.any(np.asarray(bq)) or np.any(np.asarray(bk)) or np.any(np.asarray(bv))
    )
    res = bass_utils.run_bass_kernel_spmd(
        nc=_get_nc(with_bias), in_maps=in_maps, core_ids=list(range(NC))
    )
    parts = [np.asarray(r["out"], np.float32) for r in res.results]
    return _gather(parts, bo)
